# revision 1
# baseline (speedup 1.0000x reference)
"""Trainium2 Bass kernel for nn_Block2x2DiagProduct (butterfly product).

Strategy:
  Stages 1..9 of the butterfly (all with block size <= 512) compose into
  blockdiag(R, R) with a single dense 512x512 matrix R shared by both
  halves (parameters are shared across blocks within each factor). The
  final stage (block size 1024) is a columnwise 2x2 butterfly:

      out[:, k]     = A[k]*y[:, k] + B[k]*y[:, 512+k]
      out[:, 512+k] = C[k]*y[:, k] + D[k]*y[:, 512+k]

  where y = x @ blockdiag(R^T, R^T). So the device kernel is two K=512
  bf16 matmuls per row tile (PE) plus six columnwise multiply/adds
  (Vector + GpSimd), with Scalar staging the PE-transposed x to SBUF.

  The kernel runs at the per-core DMA roofline: ~33.5 MB of HBM traffic
  (16.8 MB in + 16.8 MB out) at the measured ~415-420 GB/s combined
  ceiling of the 16 SDMA engines. Trace-driven choices:
    - x is declared float32r: PE transpose-mode streams f32r at 1.5
      cycles/row vs plain fp32's 2.0 (bit-identical storage, no cast).
    - W1t, the transposed-x staging tiles, the matmuls, the stage-0
      coefficients and stage tiles are bf16: halves the constant DMA and
      GpSimd's add bytes at ~3e-3 rel err against a 2e-2 budget.
    - Row packing "(p j) f -> p j f": each partition holds 2 consecutive
      HBM rows, so load/store descriptors are 8 KiB (vs 4 KiB), lifting
      per-DMA-engine throughput (~395 -> ~418 GB/s aggregate).
    - Startup-critical DMAs are explicitly ordered FIFO on the SP ring
      (x block 0, W1t, coef, then the prefetch flood); letting them
      round-robin on the other ring stalled the first matmuls/multiplies
      (and with them PSUM recycling) until ~30us.
    - A 26-instruction PE warmup burst runs during the first load so
      the HAM clock-gate releases 1.2 -> 2.4 GHz before real work lands.
    - The final j-tile uses narrow unfused multiplies and per-half
      stores to shorten the end-of-kernel dependency chain.
    - Software-pipelined PE stream: transposes run two tiles ahead of
      the matmuls, so PE never waits on the Scalar PSUM->SBUF staging
      copies; matmuls are h-outer so the first accumulation group only
      waits on its own half's staged transpose.
    - The two stage-0 multiplies per PSUM tile fuse into one DVE op
      via a stride-0 broadcast of the PSUM source against a contiguous
      coefficient pair (coef laid out [A,C,B,D]).
    - Steady state runs at the DMA floor: engines 100% busy at ~418
      GB/s, a ~5.0us period per 256-row block, with Tensor ~95% and
      Vector/GpSimd just below.

  R is composed on the host in float64 (9 einsums over a 512x512
  identity). Sharding: pure data parallel - batch dim of x split across
  8 cores; R^T (0.5 MiB bf16) and the stage-0 coefficients are
  replicated.
"""

import os
import sys

for _p in ("/opt/trn_rl_repo", "/root/.axon_site/_ro/trn_rl_repo"):
    if os.path.isdir(_p) and _p not in sys.path:
        sys.path.insert(0, _p)

import numpy as np

import concourse.bacc as bacc
import concourse.mybir as mybir
from concourse.bass import broadcast_tensor_aps
from concourse.bass_utils import run_bass_kernel_spmd
from concourse.masks import make_identity
from concourse.tile import TileContext

SIZE = 1024
HALF = SIZE // 2
M = 10  # number of butterfly factors
N_CORES = 8
P = 128
KC = HALF // P  # 4 contraction chunks per half
J = 2  # rows per partition per block (8 KiB contiguous HBM per partition)
BLK = P * J  # 256 rows per block

# Results of the last device run (for the test harness).
last_exec_time_ns = None
last_mean_exec_time_ns = None

_nc_cache = {}


def _compose_w1t(params):
    """Compose butterfly stages 1..9 into W1t (512x512, f64) such that
    y_half = x_half @ W1t for each 512 half. Both halves share W1t because
    each factor's parameters are shared across its blocks."""
    w = np.eye(HALF, dtype=np.float64)
    for i in reversed(range(1, M)):
        s = SIZE >> i
        y = w.reshape(HALF, HALF // s, 2, s // 2)
        w = np.einsum(
            "ijk,bnjk->bnik", params[i].astype(np.float64), y
        ).reshape(HALF, HALF)
    return w


def _build_nc(rows):
    f32 = mybir.dt.float32
    f32r = mybir.dt.float32r
    bf16 = mybir.dt.bfloat16
    nblk = rows // BLK

    # Bacc (not raw Bass): its finalize() pipeline splits multi-sem waits
    # into EventSemaphore instructions (HW allows 1 sync-wait per inst).
    nc = bacc.Bacc(None, target_bir_lowering=False)
    x_d = nc.dram_tensor("x", [rows, SIZE], f32r, kind="ExternalInput")
    w_d = nc.dram_tensor("w", [HALF, HALF], bf16, kind="ExternalInput")
    coef_d = nc.dram_tensor("coef", [P, 4, HALF], bf16, kind="ExternalInput")
    o_d = nc.dram_tensor("o", [rows, SIZE], f32, kind="ExternalOutput")

    with TileContext(nc) as tc:
        with (
            tc.tile_pool(name="const", bufs=1) as const_pool,
            tc.tile_pool(name="xin", bufs=8) as xpool,
            tc.tile_pool(name="xt", bufs=8) as xtpool,
            tc.tile_pool(name="stage", bufs=6) as spool,
            tc.tile_pool(name="osb", bufs=3) as opool,
            tc.tile_pool(name="tpsum", bufs=4, space="PSUM") as tpsum,
            tc.tile_pool(name="mpsum", bufs=4, space="PSUM") as mpsum,
        ):
            ident_f32 = const_pool.tile([P, P], f32)
            make_identity(nc, ident_f32[:])
            # GpSimd memset can't target f32r tiles, so build in f32 and
            # cast (f32r transpose needs an f32r identity operand).
            ident = const_pool.tile([P, P], f32r)
            nc.vector.tensor_copy(out=ident[:], in_=ident_f32[:])
            # PE warmup burst: the PE HAM clock-gate defaults to 1.2 GHz
            # and needs ~3.4us of sustained busy to release to 2.4 GHz.
            # The PE would otherwise sit idle until the first x load lands
            # (~12us) and then run the first blocks at half clock. These
            # no-dependency matmuls (first one doubles as the dummy
            # consuming the identity, which walrus needs so the first real
            # transpose carries a single sync-wait) run during the load
            # window and cost nothing.
            pst0 = tpsum.tile([P, P], f32r, name="pst_warm", tag="pst")
            for _ in range(32):
                nc.tensor.transpose(pst0[:], ident[:], ident[:])

            x_tiles = {}
            o_tiles = {}

            def load_block(blk, split=False):
                # Partition p holds rows blk*256 + 2p, 2p+1: 8 KiB
                # contiguous per partition -> large DMA descriptors.
                x_sb = xpool.tile([P, J, SIZE], f32r)
                src = x_d[blk * BLK : (blk + 1) * BLK, :].rearrange(
                    "(p j) f -> p j f", j=J
                )
                if split:
                    # Block 0 only: per-j loads so the first transposes wait
                    # on 512 KiB, not 1 MiB (the warmup chain is tuned to
                    # end as the first half lands).
                    for j in range(J):
                        nc.sync.dma_start(out=x_sb[:, j, :], in_=src[:, j, :])
                else:
                    nc.sync.dma_start(out=x_sb[:], in_=src)
                x_tiles[blk] = x_sb

            def emit_transposes(blk, j):
                # Transpose 8 chunks of [128b, 128f] -> [128f, 128b],
                # 4 chunks per PSUM bank, one Scalar-engine cast each.
                x_sb = x_tiles[blk]
                xts = []
                for h in range(2):
                    pst = tpsum.tile([P, HALF], f32r, tag="pst", name=f"pst{h}")
                    for c in range(KC):
                        k = KC * h + c
                        nc.tensor.transpose(
                            pst[:, c * P : (c + 1) * P],
                            x_sb[:, j, k * P : (k + 1) * P],
                            ident[:],
                        )
                    xt_h = xtpool.tile([P, HALF], bf16, tag="xt", name=f"xt{h}")
                    nc.scalar.copy(out=xt_h[:], in_=pst[:])
                    xts.append(xt_h)
                return xts

            def emit_mm_stage0(blk, j, xts):
                # y_half[b, :] = sum_k x_half[b, k] * W1t[k, :], h-outer so
                # the h=0 group starts as soon as its staging copy lands.
                o_sb = o_tiles[blk]
                psos = []
                for h in range(2):
                    pso = mpsum.tile(
                        [P, HALF], f32, tag="mm_psum", name=f"pso{h}"
                    )
                    for c in range(KC):
                        nc.tensor.matmul(
                            pso[:],
                            xts[h][:, c * P : (c + 1) * P],
                            w_sb[:, c, :],
                            start=(c == 0),
                            stop=(c == KC - 1),
                        )
                    psos.append(pso)
                # Peeled stage 0: out_lo = A*y_lo + B*y_hi, out_hi =
                # C*y_lo + D*y_hi. Vector multiplies straight from PSUM
                # (GpSimd cannot read PSUM); GpSimd adds. Each PSUM tile's
                # two multiplies fuse into ONE DVE op via a stride-0
                # broadcast of the PSUM source against a coefficient pair
                # (coef is laid out [A,C,B,D] so pairs are contiguous):
                # halves the per-op overhead. Dependencies are unchanged -
                # t02 still waits only on the h=0 accumulation group. bf16
                # stage tiles; the 2e-2 error budget dwarfs bf16's ~2e-3.
                last = blk == nblk - 1 and j == J - 1
                if last:
                    # Final j-tile: narrow unfused ops + per-half stores to
                    # shorten the end-of-kernel dependency chain.
                    o_ap = o_d[blk * BLK : (blk + 1) * BLK, :].rearrange(
                        "(p j) f -> p j f", j=J
                    )
                    t0 = spool.tile([P, HALF], bf16, tag="t02", name="t0")
                    t1 = spool.tile([P, HALF], bf16, tag="t13", name="t1")
                    t2 = spool.tile([P, HALF], bf16, tag="t02b", name="t2")
                    t3 = spool.tile([P, HALF], bf16, tag="t13b", name="t3")
                    nc.vector.tensor_mul(t0[:], psos[0][:], coef_sb[:, 0, :])
                    nc.vector.tensor_mul(t1[:], psos[1][:], coef_sb[:, 2, :])
                    nc.gpsimd.tensor_add(o_sb[:, j, :HALF], t0[:], t1[:])
                    nc.scalar.dma_start(
                        out=o_ap[:, j : j + 1, :HALF],
                        in_=o_sb[:, j : j + 1, :HALF],
                    )
                    nc.vector.tensor_mul(t2[:], psos[0][:], coef_sb[:, 1, :])
                    nc.vector.tensor_mul(t3[:], psos[1][:], coef_sb[:, 3, :])
                    # Vector is idle by now; its add avoids the GpSimd
                    # queue + cross-engine hop on the very last result.
                    nc.vector.tensor_add(o_sb[:, j, HALF:], t2[:], t3[:])
                    nc.scalar.dma_start(
                        out=o_ap[:, j : j + 1, HALF:],
                        in_=o_sb[:, j : j + 1, HALF:],
                    )
                    return
                t02 = spool.tile([P, 2, HALF], bf16, tag="t02", name="t02")
                t13 = spool.tile([P, 2, HALF], bf16, tag="t13", name="t13")
                for tt, pso, pair in ((t02, psos[0], 0), (t13, psos[1], 2)):
                    in0, in1 = broadcast_tensor_aps(
                        pso[:].rearrange("p (o f) -> p o f", o=1),
                        coef_sb[:, pair : pair + 2, :],
                    )
                    nc.vector.tensor_mul(tt[:], in0, in1)
                nc.gpsimd.tensor_add(
                    o_sb[:, j, :HALF], t02[:, 0, :], t13[:, 0, :]
                )
                nc.gpsimd.tensor_add(
                    o_sb[:, j, HALF:], t02[:, 1, :], t13[:, 1, :]
                )
                # Store a j-pair as soon as it completes: rows 2p, 2p+1
                # are HBM-adjacent, giving 8 KiB store descriptors on the
                # ACT queue (separate from the SP load queue).
                if blk == nblk - 1:
                    # Final block: store each j-tile as it completes so the
                    # kernel tail isn't gated on the whole block.
                    nc.scalar.dma_start(
                        out=o_d[blk * BLK : (blk + 1) * BLK, :].rearrange(
                            "(p j) f -> p j f", j=J
                        )[:, j : j + 1, :],
                        in_=o_sb[:, j : j + 1, :],
                    )
                elif j % 2 == 1:
                    nc.scalar.dma_start(
                        out=o_d[blk * BLK : (blk + 1) * BLK, :].rearrange(
                            "(p j) f -> p j f", j=J
                        )[:, j - 1 : j + 1, :],
                        in_=o_sb[:, j - 1 : j + 1, :],
                    )

            # Startup-critical DMA ordering, all FIFO on the SP ring:
            # x block 0 first (gates the first transposes), then the W1t
            # chunks (gate the first matmuls), then coef (gates the first
            # stage-0 multiplies), then the prefetch flood. When w/coef sat
            # on the ACT ring they round-robined against the 6 MiB prefetch
            # and straggled to ~31us, stalling the whole stage-0 chain.
            load_block(0)
            # W1t and the matmuls run in bf16: halves the W and coef DMA
            # (startup is load-latency-bound), enables FWL on LDWEIGHTS, and
            # the 2e-2 error budget dwarfs bf16's ~2e-3.
            w_sb = const_pool.tile([P, KC, HALF], bf16)
            for c in range(KC):
                nc.sync.dma_start(
                    out=w_sb[:, c, :], in_=w_d[c * P : (c + 1) * P, :]
                )
            coef_sb = const_pool.tile([P, 4, HALF], bf16)
            nc.sync.dma_start(out=coef_sb[:], in_=coef_d[:, :, :])
            # Prefetch depth 8 blocks (matches xpool bufs).
            for blk in range(1, min(8, nblk)):
                load_block(blk)

            # Software pipeline: transposes run TWO tiles ahead of the
            # matmuls (PE stream T(0) T(1) T(2) MM(0) T(3) MM(1) ...), so
            # the Scalar staging copies have a full tile of slack and the
            # matmuls never wait on them at block boundaries.
            tiles = [(blk, j) for blk in range(nblk) for j in range(J)]
            pending = []
            for blk, j in tiles:
                if j == 0:
                    o_tiles[blk] = opool.tile([P, J, SIZE], f32, name="o_sb")
                    if blk + 8 < nblk:
                        load_block(blk + 8)
                pending.append((blk, j, emit_transposes(blk, j)))
                if len(pending) > 2:
                    pb, pj, pxts = pending.pop(0)
                    emit_mm_stage0(pb, pj, pxts)
                    if pj == J - 1:
                        del x_tiles[pb], o_tiles[pb]
            for pb, pj, pxts in pending:
                emit_mm_stage0(pb, pj, pxts)
                if pj == J - 1:
                    del x_tiles[pb], o_tiles[pb]
    nc.finalize()
    return nc


def kernel(**inputs):
    global last_exec_time_ns, last_mean_exec_time_ns

    x = np.ascontiguousarray(np.asarray(inputs["x"], dtype=np.float32))
    params = [np.asarray(inputs[f"ABCD{i}"]) for i in range(M)]
    bf16_np = mybir.dt.np(mybir.dt.bfloat16)
    w1t = np.ascontiguousarray(_compose_w1t(params).astype(bf16_np))
    abcd = params[0].astype(bf16_np)  # (2, 2, 512): [[A, B], [C, D]]
    acbd = np.stack([abcd[0, 0], abcd[1, 0], abcd[0, 1], abcd[1, 1]])
    coef = np.ascontiguousarray(
        np.broadcast_to(acbd.reshape(1, 4, HALF), (P, 4, HALF))
    )

    batch = x.shape[0]
    if batch % (N_CORES * BLK) != 0:
        # Shape outside the tiled layout this kernel hardcodes - fall back
        # to a host matmul (correct, just not accelerated).
        full = _compose_w1t(params)
        y_lo = x[:, :HALF].astype(np.float64) @ full
        y_hi = x[:, HALF:].astype(np.float64) @ full
        a, b = params[0][0, 0].astype(np.float64), params[0][0, 1].astype(
            np.float64
        )
        c, dd = params[0][1, 0].astype(np.float64), params[0][1, 1].astype(
            np.float64
        )
        return np.concatenate(
            [a * y_lo + b * y_hi, c * y_lo + dd * y_hi], axis=1
        ).astype(np.float32)
    rows = batch // N_CORES

    if rows not in _nc_cache:
        _nc_cache[rows] = _build_nc(rows)
    nc = _nc_cache[rows]

    in_maps = [
        {"x": x[i * rows : (i + 1) * rows], "w": w1t, "coef": coef}
        for i in range(N_CORES)
    ]
    try:
        res = run_bass_kernel_spmd(nc, in_maps, core_ids=list(range(N_CORES)))
    except Exception:
        # Transient axon/PJRT INTERNAL errors have been observed on the
        # first attempt in a fresh process; one retry clears them.
        res = run_bass_kernel_spmd(nc, in_maps, core_ids=list(range(N_CORES)))
    last_exec_time_ns = res.exec_time_ns
    last_mean_exec_time_ns = res.mean_exec_time_ns

    return np.concatenate([r["o"] for r in res.results], axis=0)

